# revision 13
# baseline (speedup 1.0000x reference)
"""Trainium2 Bass kernel for nn_CustomTripletLoss (B=16384, C=1000, D=1024).

Strategy (data-parallel over the anchor dim, 8 cores x 2048 anchors):
  For each anchor b:  d2[b, c] = |x_b|^2 - 2<x_b, t_c> + |t_c|^2
  The loss needs   d2_ap = d2[b, label_b]   and   d2_an = min_{c != label} d2[b, c].
  Per core we compute  Q[b, c] = 2<x_b, t_c> - |t_c|^2 + BIG*(c == label_b)
  entirely on the tensor engine (float32r matmuls, x transposed on-chip by the
  PE, |t|^2 folded in as a K=1 matmul, the one-hot mask folded in by the DVE
  during the PSUM->SBUF move).  vector.max then gives the top-8 of each row:
    top0 = Q[b, label_b] + BIG   (BIG dominates)  ->  d2_ap = x2 + BIG - top0
    top1 = max_{c != label} Q    ->  d2_an = x2 - top1
  The kernel exports top-8 rows and |x_b|^2; the host finishes with
  sqrt/hinge/mean over the 16384 anchors (in float64) and returns the scalar.
"""

import numpy as np

import concourse.bass as bass
import concourse.tile as tile
from concourse import bacc, mybir
from concourse.bass_utils import run_bass_kernel_spmd
from concourse.masks import make_identity

B, C, D = 16384, 1000, 1024
N_CORES = 8
BS = B // N_CORES          # 2048 anchors per core
NT = BS // 128             # 16 b-tiles per core
KT = D // 128              # 8 contraction chunks
CT = (C + 127) // 128      # 8 target row-tiles (last one 104 rows)
HALF = 500                 # free-dim half (one PSUM bank each, <=512)
BIG = 4096.0               # one-hot bias; > max-min spread of Q (~1300)

F32 = mybir.dt.float32
F32R = mybir.dt.float32r


def _r(ap):
    return ap.bitcast(F32R)


def build_program():
    nc = bacc.Bacc("TRN2", target_bir_lowering=False, debug=False)

    x_d = nc.dram_tensor("inputs", [BS, D], F32, kind="ExternalInput").ap()
    t_d = nc.dram_tensor("target", [C, D], F32, kind="ExternalInput").ap()
    lab_d = nc.dram_tensor("labels_f", [BS], F32, kind="ExternalInput").ap()
    omax_d = nc.dram_tensor("out_max8", [128, NT * 8], F32, kind="ExternalOutput").ap()
    ox2_d = nc.dram_tensor("out_x2", [128, NT], F32, kind="ExternalOutput").ap()

    with tile.TileContext(nc) as tc:
        with (
            tc.tile_pool(name="consts", bufs=1) as consts,
            tc.tile_pool(name="tmat", bufs=1) as tmat,
            tc.tile_pool(name="sb", bufs=3) as sb,
            tc.tile_pool(name="outp", bufs=1) as outp,
            tc.tile_pool(name="psum", bufs=2, space="PSUM") as psum,
        ):
            # ---- constants -------------------------------------------------
            ident = consts.tile([128, 128], F32)
            make_identity(nc, ident)

            ones_row = consts.tile([1, 128], F32R)
            nc.vector.memset(ones_row.bitcast(F32), 1.0)
            ones_col = consts.tile([128, 1], F32R)
            nc.vector.memset(ones_col.bitcast(F32), 1.0)

            iota_f = consts.tile([128, C], F32)
            nc.gpsimd.iota(
                iota_f,
                pattern=[[1, C]],
                base=0,
                channel_multiplier=0,
                allow_small_or_imprecise_dtypes=True,
            )

            lab_sb = consts.tile([128, NT], F32)
            nc.sync.dma_start(lab_sb, lab_d.rearrange("(i p) -> p i", p=128))
            # Drain the DMA/Pool waits into DVE-side copies so the
            # TensorScalarPtr mask builds below need no sync-wait commands
            # (the TS ISA struct has too few wait slots).
            lab_v = consts.tile([128, NT], F32)
            nc.vector.tensor_copy(lab_v, lab_sb)
            iota_warm = consts.tile([128, 8], F32)
            nc.vector.tensor_copy(iota_warm, iota_f[:, :8])

            # ---- target transpose: tT[:, k, c] = target[c, 128k + dpart] ---
            tT = tmat.tile([128, KT, C], F32R)
            for j in range(CT):
                cs = min(128, C - j * 128)
                t_str = sb.tile([128, D], F32, tag="tload")
                nc.sync.dma_start(t_str[:cs], t_d[j * 128 : j * 128 + cs, :])
                pt = psum.tile([128, KT, 128], F32, tag="xt")
                for k in range(KT):
                    nc.tensor.transpose(
                        pt[:, k, :cs],
                        t_str[:cs, k * 128 : (k + 1) * 128],
                        ident[:cs, :cs],
                    )
                nc.scalar.copy(tT[:, :, j * 128 : j * 128 + cs], pt[:, :, :cs])

            # ---- t2neg row: -|t_c|^2 as [1, C] ----------------------------
            t2_ps = psum.tile([128, 2, 512], F32, tag="q")
            for k in range(KT):
                tsq = sb.tile([128, C], F32R, tag="tsq")
                nc.scalar.activation(
                    tsq, tT[:, k, :].bitcast(F32), mybir.ActivationFunctionType.Square
                )
                for h in range(2):
                    nc.tensor.matmul(
                        t2_ps[:1, h, :HALF],
                        lhsT=ones_col,
                        rhs=tsq[:, h * HALF : (h + 1) * HALF],
                        start=(k == 0),
                        stop=(k == KT - 1),
                    )
            t2neg = consts.tile([1, C], F32R)
            nc.scalar.mul(
                t2neg.rearrange("p (h c) -> p h c", h=2), t2_ps[:1, :, :HALF], -1.0
            )

            # ---- outputs ---------------------------------------------------
            max8_sb = outp.tile([128, NT * 8], F32)
            x2cols = outp.tile([128, NT], F32)

            # ---- main loop over 16 b-tiles --------------------------------
            for i in range(NT):
                x_t = sb.tile([128, D], F32, tag="x")
                nc.sync.dma_start(x_t, x_d[i * 128 : (i + 1) * 128, :])

                # |x|^2 per anchor (ACT square + free-dim accumulate)
                xsq = sb.tile([128, D], F32, tag="xsq")
                nc.scalar.activation(
                    xsq,
                    x_t,
                    mybir.ActivationFunctionType.Square,
                    accum_out=x2cols[:, i : i + 1],
                )

                # one-hot mask * BIG (DVE tensor_scalar, two fused ops)
                m_eq = sb.tile([128, C], F32, tag="m")
                nc.vector.tensor_scalar(
                    m_eq,
                    iota_f,
                    lab_v[:, i : i + 1],
                    BIG,
                    mybir.AluOpType.is_equal,
                    mybir.AluOpType.mult,
                )

                # transpose x tile: pxt[:, k, b] = x[b, 128k + dpart]
                pxt = psum.tile([128, KT, 128], F32, tag="xt")
                for k in range(KT):
                    nc.tensor.transpose(
                        pxt[:, k, :], x_t[:, k * 128 : (k + 1) * 128], ident
                    )
                xt2 = sb.tile([128, KT, 128], F32R, tag="xt2")
                nc.scalar.mul(xt2, pxt, 2.0)  # xt2 = 2 * x^T chunks

                # Q = 2 x t^T - t2  (accumulated in PSUM, two 500-wide banks)
                q_ps = psum.tile([128, 2, 512], F32, tag="q")
                for k in range(KT):
                    for h in range(2):
                        nc.tensor.matmul(
                            q_ps[:, h, :HALF],
                            lhsT=xt2[:, k, :],
                            rhs=tT[:, k, h * HALF : (h + 1) * HALF],
                            start=(k == 0),
                            stop=False,
                        )
                for h in range(2):
                    nc.tensor.matmul(
                        q_ps[:, h, :HALF],
                        lhsT=ones_row,
                        rhs=t2neg[:, h * HALF : (h + 1) * HALF],
                        start=False,
                        stop=True,
                    )

                # Qs = M + Q   (PSUM -> SBUF move with mask folded in)
                qs = sb.tile([128, C], F32, tag="qs")
                for h in range(2):
                    nc.vector.scalar_tensor_tensor(
                        qs[:, h * HALF : (h + 1) * HALF],
                        m_eq[:, h * HALF : (h + 1) * HALF],
                        1.0,
                        q_ps[:, h, :HALF],
                        mybir.AluOpType.mult,
                        mybir.AluOpType.add,
                    )

                # top-8 of each row
                nc.vector.max(max8_sb[:, i * 8 : (i + 1) * 8], qs)

            nc.sync.dma_start(omax_d, max8_sb)
            nc.sync.dma_start(ox2_d, x2cols)

    nc.compile()
    return nc


_NC_CACHE = None


def _get_nc():
    global _NC_CACHE
    if _NC_CACHE is None:
        _NC_CACHE = build_program()
    return _NC_CACHE


def _postprocess(results):
    total = 0.0
    for c in range(N_CORES):
        m8 = np.asarray(results[c]["out_max8"], dtype=np.float64).reshape(128, NT, 8)
        x2 = np.asarray(results[c]["out_x2"], dtype=np.float64)  # [128, NT]
        top0 = m8[..., 0]
        top1 = m8[..., 1]
        d2_ap = np.maximum(x2 - (top0 - BIG), 0.0)
        d2_an = np.maximum(x2 - top1, 0.0)
        per = np.maximum(np.sqrt(d2_ap) - np.sqrt(d2_an) + 1.0, 0.0)
        total += per.sum()
    return np.float32(total / B)


def run(inputs, labels, target, trace=False):
    nc = _get_nc()
    x = np.ascontiguousarray(np.asarray(inputs, dtype=np.float32))
    t = np.ascontiguousarray(np.asarray(target, dtype=np.float32))
    lab = np.ascontiguousarray(np.asarray(labels).astype(np.float32))
    assert x.shape == (B, D) and t.shape == (C, D) and lab.shape == (B,)

    in_maps = [
        {
            "inputs": x[c * BS : (c + 1) * BS],
            "labels_f": lab[c * BS : (c + 1) * BS],
            "target": t,
        }
        for c in range(N_CORES)
    ]
    res = run_bass_kernel_spmd(nc, in_maps, list(range(N_CORES)), trace=trace)
    return _postprocess(res.results), res


def kernel(inputs, labels, target):
    out, _ = run(inputs, labels, target)
    return out


# revision 15
# speedup vs baseline: 40.0224x; 40.0224x over previous
"""Trainium2 Bass kernel for nn_CustomTripletLoss (B=16384, C=1000, D=1024).

Strategy (data-parallel over the anchor dim, 8 cores x 2048 anchors):
  For each anchor b:  d2[b, c] = |x_b|^2 - 2<x_b, t_c> + |t_c|^2
  The loss needs   d2_ap = d2[b, label_b]   and   d2_an = min_{c != label} d2[b, c].
  Per core we compute  Q[b, c] = 2<x_b, t_c> - |t_c|^2 + BIG*(c == label_b)
  entirely on the tensor engine (float32r matmuls, x transposed on-chip by the
  PE, |t|^2 folded in as a K=1 matmul, the one-hot mask folded in by the DVE
  during the PSUM->SBUF move).  vector.max then gives the top-8 of each row:
    top0 = Q[b, label_b] + BIG   (BIG dominates)  ->  d2_ap = x2 + BIG - top0
    top1 = max_{c != label} Q    ->  d2_an = x2 - top1
  The kernel exports top-8 rows and |x_b|^2; the host finishes with
  sqrt/hinge/mean over the 16384 anchors (in float64) and returns the scalar.
"""

import numpy as np

import concourse.bass as bass
import concourse.tile as tile
from concourse import bacc, mybir
from concourse.bass_utils import run_bass_kernel_spmd
from concourse.masks import make_identity

B, C, D = 16384, 1000, 1024
N_CORES = 8
BS = B // N_CORES          # 2048 anchors per core
NT = BS // 128             # 16 b-tiles per core
KT = D // 128              # 8 contraction chunks
CT = (C + 127) // 128      # 8 target row-tiles (last one 104 rows)
HALF = 500                 # free-dim half (one PSUM bank each, <=512)
BIG = 4096.0               # one-hot bias; > max-min spread of Q (~1300)

F32 = mybir.dt.float32
F32R = mybir.dt.float32r


def _r(ap):
    return ap.bitcast(F32R)


def build_program(repeat=1):
    """repeat>1 re-runs the main loop (same data, same outputs) so device-side
    per-pass time can be extracted by differencing two repeat counts."""
    nc = bacc.Bacc("TRN2", target_bir_lowering=False, debug=False)

    x_d = nc.dram_tensor("inputs", [BS, D], F32, kind="ExternalInput").ap()
    t_d = nc.dram_tensor("target", [C, D], F32, kind="ExternalInput").ap()
    lab_d = nc.dram_tensor("labels_f", [BS], F32, kind="ExternalInput").ap()
    omax_d = nc.dram_tensor("out_max8", [128, NT * 8], F32, kind="ExternalOutput").ap()
    ox2_d = nc.dram_tensor("out_x2", [128, NT], F32, kind="ExternalOutput").ap()

    with tile.TileContext(nc) as tc:
        with (
            tc.tile_pool(name="consts", bufs=1) as consts,
            tc.tile_pool(name="tmat", bufs=1) as tmat,
            tc.tile_pool(name="sb", bufs=3) as sb,
            tc.tile_pool(name="outp", bufs=1) as outp,
            tc.tile_pool(name="psum", bufs=2, space="PSUM") as psum,
        ):
            # ---- constants -------------------------------------------------
            ident = consts.tile([128, 128], F32)
            make_identity(nc, ident)

            ones_row = consts.tile([1, 128], F32R)
            nc.vector.memset(ones_row.bitcast(F32), 1.0)
            ones_col = consts.tile([128, 1], F32R)
            nc.vector.memset(ones_col.bitcast(F32), 1.0)

            iota_f = consts.tile([128, C], F32)
            nc.gpsimd.iota(
                iota_f,
                pattern=[[1, C]],
                base=0,
                channel_multiplier=0,
                allow_small_or_imprecise_dtypes=True,
            )

            lab_sb = consts.tile([128, NT], F32)
            nc.sync.dma_start(lab_sb, lab_d.rearrange("(i p) -> p i", p=128))
            # Drain the DMA/Pool waits into DVE-side copies so the
            # TensorScalarPtr mask builds below need no sync-wait commands
            # (the TS ISA struct has too few wait slots).
            lab_v = consts.tile([128, NT], F32)
            nc.vector.tensor_copy(lab_v, lab_sb)
            iota_warm = consts.tile([128, 8], F32)
            nc.vector.tensor_copy(iota_warm, iota_f[:, :8])

            # ---- target transpose: tT[:, k, c] = target[c, 128k + dpart] ---
            tT = tmat.tile([128, KT, C], F32R)
            for j in range(CT):
                cs = min(128, C - j * 128)
                t_str = sb.tile([128, D], F32, tag="tload")
                nc.sync.dma_start(t_str[:cs], t_d[j * 128 : j * 128 + cs, :])
                pt = psum.tile([128, KT, 128], F32, tag="xt")
                for k in range(KT):
                    nc.tensor.transpose(
                        pt[:, k, :cs],
                        t_str[:cs, k * 128 : (k + 1) * 128],
                        ident[:cs, :cs],
                    )
                nc.scalar.copy(tT[:, :, j * 128 : j * 128 + cs], pt[:, :, :cs])

            # ---- t2neg row: -|t_c|^2 as [1, C] ----------------------------
            t2_ps = psum.tile([128, 2, 512], F32, tag="q")
            for k in range(KT):
                tsq = sb.tile([128, C], F32R, tag="tsq")
                nc.scalar.activation(
                    tsq, tT[:, k, :].bitcast(F32), mybir.ActivationFunctionType.Square
                )
                for h in range(2):
                    nc.tensor.matmul(
                        t2_ps[:1, h, :HALF],
                        lhsT=ones_col,
                        rhs=tsq[:, h * HALF : (h + 1) * HALF],
                        start=(k == 0),
                        stop=(k == KT - 1),
                    )
            t2neg = consts.tile([1, C], F32R)
            nc.scalar.mul(
                t2neg.rearrange("p (h c) -> p h c", h=2), t2_ps[:1, :, :HALF], -1.0
            )

            # ---- outputs ---------------------------------------------------
            max8_sb = outp.tile([128, NT * 8], F32)
            x2cols = outp.tile([128, NT], F32)

            # ---- main loop over 16 b-tiles --------------------------------
            for ii in range(NT * repeat):
                i = ii % NT
                x_t = sb.tile([128, D], F32, tag="x")
                nc.sync.dma_start(x_t, x_d[i * 128 : (i + 1) * 128, :])

                # |x|^2 per anchor (ACT square + free-dim accumulate)
                xsq = sb.tile([128, D], F32, tag="xsq")
                nc.scalar.activation(
                    xsq,
                    x_t,
                    mybir.ActivationFunctionType.Square,
                    accum_out=x2cols[:, i : i + 1],
                )

                # one-hot mask * BIG (DVE tensor_scalar, two fused ops)
                m_eq = sb.tile([128, C], F32, tag="m")
                nc.vector.tensor_scalar(
                    m_eq,
                    iota_f,
                    lab_v[:, i : i + 1],
                    BIG,
                    mybir.AluOpType.is_equal,
                    mybir.AluOpType.mult,
                )

                # transpose x tile: pxt[:, k, b] = x[b, 128k + dpart]
                pxt = psum.tile([128, KT, 128], F32, tag="xt")
                for k in range(KT):
                    nc.tensor.transpose(
                        pxt[:, k, :], x_t[:, k * 128 : (k + 1) * 128], ident
                    )
                xt2 = sb.tile([128, KT, 128], F32R, tag="xt2")
                nc.scalar.mul(xt2, pxt, 2.0)  # xt2 = 2 * x^T chunks

                # Q = 2 x t^T - t2  (accumulated in PSUM, two 500-wide banks)
                q_ps = psum.tile([128, 2, 512], F32, tag="q")
                for k in range(KT):
                    for h in range(2):
                        nc.tensor.matmul(
                            q_ps[:, h, :HALF],
                            lhsT=xt2[:, k, :],
                            rhs=tT[:, k, h * HALF : (h + 1) * HALF],
                            start=(k == 0),
                            stop=False,
                        )
                for h in range(2):
                    nc.tensor.matmul(
                        q_ps[:, h, :HALF],
                        lhsT=ones_row,
                        rhs=t2neg[:, h * HALF : (h + 1) * HALF],
                        start=False,
                        stop=True,
                    )

                # Qs = M + Q   (PSUM -> SBUF move with mask folded in)
                qs = sb.tile([128, C], F32, tag="qs")
                for h in range(2):
                    nc.vector.scalar_tensor_tensor(
                        qs[:, h * HALF : (h + 1) * HALF],
                        m_eq[:, h * HALF : (h + 1) * HALF],
                        1.0,
                        q_ps[:, h, :HALF],
                        mybir.AluOpType.mult,
                        mybir.AluOpType.add,
                    )

                # top-8 of each row
                nc.vector.max(max8_sb[:, i * 8 : (i + 1) * 8], qs)

            nc.sync.dma_start(omax_d, max8_sb)
            nc.sync.dma_start(ox2_d, x2cols)

    nc.compile()
    return nc


_NC_CACHE = None


def _get_nc():
    global _NC_CACHE
    if _NC_CACHE is None:
        _NC_CACHE = build_program()
    return _NC_CACHE


def _postprocess(results):
    total = 0.0
    for c in range(N_CORES):
        m8 = np.asarray(results[c]["out_max8"], dtype=np.float64).reshape(128, NT, 8)
        x2 = np.asarray(results[c]["out_x2"], dtype=np.float64)  # [128, NT]
        top0 = m8[..., 0]
        top1 = m8[..., 1]
        d2_ap = np.maximum(x2 - (top0 - BIG), 0.0)
        d2_an = np.maximum(x2 - top1, 0.0)
        per = np.maximum(np.sqrt(d2_ap) - np.sqrt(d2_an) + 1.0, 0.0)
        total += per.sum()
    return np.float32(total / B)


def run(inputs, labels, target, trace=False):
    nc = _get_nc()
    x = np.ascontiguousarray(np.asarray(inputs, dtype=np.float32))
    t = np.ascontiguousarray(np.asarray(target, dtype=np.float32))
    lab = np.ascontiguousarray(np.asarray(labels).astype(np.float32))
    assert x.shape == (B, D) and t.shape == (C, D) and lab.shape == (B,)

    in_maps = [
        {
            "inputs": x[c * BS : (c + 1) * BS],
            "labels_f": lab[c * BS : (c + 1) * BS],
            "target": t,
        }
        for c in range(N_CORES)
    ]
    res = run_bass_kernel_spmd(nc, in_maps, list(range(N_CORES)), trace=trace)
    return _postprocess(res.results), res


def kernel(inputs, labels, target):
    out, _ = run(inputs, labels, target)
    return out
